# revision 1
# baseline (speedup 1.0000x reference)
"""SNN leaky integrate-and-fire kernel for Trainium2 (8 NeuronCores, SPMD).

Computes, for x [30, 8192, 784] f32 and W [10, 784] f32:
    w_q  = 16-bit fixed-point quantized W (Q3.12, straight-through)
    cur  = einsum('tbi,oi->tbo', x, w_q)                  [30, 8192, 10]
    scan over t: mem_t = BETA*mem_{t-1} + cur_t - spk_{t-1); spk_t = mem_t > 1
Returns (spk_rec, mem_rec), each [30, 8192, 10] f32.

Sharding: pure data parallel over the batch axis (1024 rows per core).
Per core the input is pre-transposed on host to xt [784, 30*1024] so the
contraction axis (784) lies on SBUF partitions with fully contiguous DMA.
"""

import numpy as np

import concourse.bass as bass
import concourse.mybir as mybir
from concourse import bacc
import concourse.tile as tile
from concourse.bass_utils import run_bass_kernel_spmd

N_CORES = 8
T = 30
B = 8192
I = 784
O = 10
BC = B // N_CORES          # 1024 batch rows per core
N = T * BC                 # 30720 columns per core (t-major, then b)
F = 2048                   # column tile width (= 2 timesteps)
NT = N // F                # 15 column tiles
I_MAIN = 768               # 6 chunks of 128
I_TAIL = I - I_MAIN        # 16
BETA = 0.9375
THRESHOLD = 1.0

f32 = mybir.dt.float32


def _build_nc():
    AL = mybir.AluOpType
    nc = bacc.Bacc("TRN2")
    xt = nc.dram_tensor("xt", [I, N], f32, kind="ExternalInput")
    wt = nc.dram_tensor("wt", [I, O], f32, kind="ExternalInput")
    spk = nc.dram_tensor("spk", [O, N], f32, kind="ExternalOutput")
    mem = nc.dram_tensor("mem", [O, N], f32, kind="ExternalOutput")

    with tile.TileContext(nc) as tc:
        with (
            tc.tile_pool(name="xmain", bufs=2) as xmain_pool,
            tc.tile_pool(name="xtail", bufs=2) as xtail_pool,
            tc.tile_pool(name="cur", bufs=3) as cur_pool,
            tc.tile_pool(name="outs", bufs=4) as out_pool,
            tc.tile_pool(name="state", bufs=1) as state_pool,
            tc.tile_pool(name="psum", bufs=8, space="PSUM") as psum_pool,
        ):
            # Stationary weights: wt_sb[:, c, :] is the [128, 10] lhsT for
            # contraction chunk c; chunk 6 uses only the first 16 partitions.
            wt_sb = state_pool.tile([128, 7, O], f32)
            for c in range(6):
                nc.sync.dma_start(out=wt_sb[:, c, :], in_=wt[c * 128:(c + 1) * 128, :])
            nc.sync.dma_start(out=wt_sb[:I_TAIL, 6, :], in_=wt[I_MAIN:, :])

            mem_state = state_pool.tile([O, BC], f32)
            nc.vector.memset(mem_state, 0.0)
            spk_zero = state_pool.tile([O, BC], f32)
            nc.vector.memset(spk_zero, 0.0)
            spk_prev = spk_zero

            xt_main = xt[0:I_MAIN, :].rearrange("(c p) n -> p c n", p=128)
            t_idx = 0
            for j in range(NT):
                xm = xmain_pool.tile([128, 6, F], f32)
                nc.sync.dma_start(out=xm, in_=xt_main[:, :, j * F:(j + 1) * F])
                xtl = xtail_pool.tile([I_TAIL, F], f32)
                nc.sync.dma_start(out=xtl, in_=xt[I_MAIN:, j * F:(j + 1) * F])

                cur = cur_pool.tile([O, F], f32)
                for n0 in range(0, F, 512):
                    ps = psum_pool.tile([O, 512], f32)
                    for c in range(6):
                        nc.tensor.matmul(
                            ps,
                            lhsT=wt_sb[:, c, :],
                            rhs=xm[:, c, n0:n0 + 512],
                            start=(c == 0),
                            stop=False,
                        )
                    nc.tensor.matmul(
                        ps,
                        lhsT=wt_sb[:I_TAIL, 6, :],
                        rhs=xtl[:, n0:n0 + 512],
                        start=False,
                        stop=True,
                    )
                    nc.scalar.copy(cur[:, n0:n0 + 512], ps)

                for tt in range(F // BC):
                    sl = cur[:, tt * BC:(tt + 1) * BC]
                    # mem = (mem * BETA) + cur_t   (same op order as reference)
                    nc.vector.scalar_tensor_tensor(
                        out=mem_state, in0=mem_state, scalar=BETA, in1=sl,
                        op0=AL.mult, op1=AL.add,
                    )
                    # mem -= spk_{t-1} * THRESHOLD (THRESHOLD == 1)
                    nc.vector.tensor_sub(mem_state, mem_state, spk_prev)
                    spk_new = out_pool.tile([O, BC], f32, tag="spk")
                    nc.vector.tensor_scalar(
                        out=spk_new, in0=mem_state, scalar1=THRESHOLD,
                        scalar2=None, op0=AL.is_gt,
                    )
                    mem_out = out_pool.tile([O, BC], f32, tag="memout")
                    nc.scalar.copy(mem_out, mem_state)
                    nc.scalar.dma_start(out=spk[:, t_idx * BC:(t_idx + 1) * BC], in_=spk_new)
                    nc.scalar.dma_start(out=mem[:, t_idx * BC:(t_idx + 1) * BC], in_=mem_out)
                    spk_prev = spk_new
                    t_idx += 1

    nc.finalize()
    return nc


_NC = None


def _get_nc():
    global _NC
    if _NC is None:
        _NC = _build_nc()
    return _NC


def _quantize_w(W):
    W32 = np.asarray(W, dtype=np.float32)
    q = np.round(W32 * np.float32(4096.0))
    q = np.clip(q, np.float32(-32768.0), np.float32(32767.0)) / np.float32(4096.0)
    # straight-through forward value, replicated bit-exactly: w + (q - w)
    return (W32 + (q - W32)).astype(np.float32)


def kernel(x, W, _run_opts=None):
    x = np.asarray(x, dtype=np.float32)
    W = np.asarray(W, dtype=np.float32)
    assert x.shape == (T, B, I) and W.shape == (O, I)

    wt = np.ascontiguousarray(_quantize_w(W).T)  # [784, 10]

    in_maps = []
    for c in range(N_CORES):
        xc = x[:, c * BC:(c + 1) * BC, :]                      # [30, 1024, 784]
        xt_c = np.ascontiguousarray(xc.transpose(2, 0, 1))     # [784, 30, 1024]
        in_maps.append({"xt": xt_c.reshape(I, N), "wt": wt})

    nc = _get_nc()
    run_opts = dict(_run_opts or {})
    res = run_bass_kernel_spmd(nc, in_maps, core_ids=list(range(N_CORES)), **run_opts)

    spk_full = np.empty((T, B, O), dtype=np.float32)
    mem_full = np.empty((T, B, O), dtype=np.float32)
    for c in range(N_CORES):
        s = res.results[c]["spk"].reshape(O, T, BC).transpose(1, 2, 0)
        m = res.results[c]["mem"].reshape(O, T, BC).transpose(1, 2, 0)
        spk_full[:, c * BC:(c + 1) * BC, :] = s
        mem_full[:, c * BC:(c + 1) * BC, :] = m

    if _run_opts is not None:
        kernel.last_result = res
    return spk_full, mem_full


# revision 2
# speedup vs baseline: 1.3582x; 1.3582x over previous
"""SNN leaky integrate-and-fire kernel for Trainium2 (8 NeuronCores, SPMD).

Computes, for x [30, 8192, 784] f32 and W [10, 784] f32:
    w_q  = 16-bit fixed-point quantized W (Q3.12, straight-through)
    cur  = einsum('tbi,oi->tbo', x, w_q)                  [30, 8192, 10]
    scan over t: mem_t = BETA*mem_{t-1} + cur_t - spk_{t-1}; spk_t = mem_t > 1
Returns (spk_rec, mem_rec), each [30, 8192, 10] f32.

Sharding: pure data parallel over the batch axis (1024 rows per core).
Per core the input is pre-transposed on host to xt [784, 30*1024] so the
contraction axis (784) lies on SBUF partitions with fully contiguous DMA.

The matmul uses 4x column-group tiling of the PE array: the per-core batch
is split into 4 quarters of 256; quarter j's outputs live on partitions
32j..32j+9 (weights are zero-padded to M=32 so partitions 32j+10..32j+31
hold clean zeros).  Four matmuls (one per column group) run concurrently,
which quadruples fp32 matmul throughput.  The membrane scan runs directly
on the [128, 256] layout.
"""

import numpy as np

import concourse.bass as bass
import concourse.mybir as mybir
from concourse import bacc
import concourse.tile as tile
from concourse.bass_utils import run_bass_kernel_spmd

N_CORES = 8
T = 30
B = 8192
I = 784
O = 10
BC = B // N_CORES          # 1024 batch rows per core
BQ = BC // 4               # 256 batch rows per column group
N = T * BC                 # 30720 columns per core (t-major, then b)
F = 2 * BC                 # x column tile = 2 timesteps
I_MAIN = 768               # 6 contraction chunks of 128
I_TAIL = I - I_MAIN        # 16
KT = 5                     # timesteps per output staging flush
BETA = 0.9375
THRESHOLD = 1.0

f32 = mybir.dt.float32


def _build_nc():
    AL = mybir.AluOpType
    nc = bacc.Bacc("TRN2")
    xt = nc.dram_tensor("xt", [I, N], f32, kind="ExternalInput")
    wt = nc.dram_tensor("wt", [I, O], f32, kind="ExternalInput")
    spk = nc.dram_tensor("spk", [4, O, T, BQ], f32, kind="ExternalOutput")
    mem = nc.dram_tensor("mem", [4, O, T, BQ], f32, kind="ExternalOutput")

    with tile.TileContext(nc) as tc:
        with (
            tc.tile_pool(name="xmain", bufs=3) as xmain_pool,
            tc.tile_pool(name="xtail", bufs=3) as xtail_pool,
            tc.tile_pool(name="stage", bufs=2) as stage_pool,
            tc.tile_pool(name="state", bufs=1) as state_pool,
            tc.tile_pool(name="psum", bufs=8, space="PSUM") as psum_pool,
        ):
            # Stationary weights, zero-padded to M=32: wt_sb[:, c, :] is the
            # [128, 32] lhsT for contraction chunk c (chunk 6: 16 partitions).
            wt_sb = state_pool.tile([128, 7, 32], f32)
            nc.vector.memset(wt_sb, 0.0)
            for c in range(6):
                nc.sync.dma_start(out=wt_sb[:, c, 0:O], in_=wt[c * 128:(c + 1) * 128, :])
            nc.sync.dma_start(out=wt_sb[:I_TAIL, 6, 0:O], in_=wt[I_MAIN:, :])

            mem_state = state_pool.tile([128, BQ], f32)
            nc.vector.memset(mem_state, 0.0)
            spk_zero = state_pool.tile([128, BQ], f32)
            nc.vector.memset(spk_zero, 0.0)
            spk_prev = spk_zero

            xt_main = xt[0:I_MAIN, :].rearrange("(c p) n -> p c n", p=128)

            xm = xtl = None
            spk_stage = mem_stage = None
            for t in range(T):
                if t % 2 == 0:
                    xm = xmain_pool.tile([128, 6, F], f32)
                    nc.sync.dma_start(out=xm, in_=xt_main[:, :, t * BC:(t + 2) * BC])
                    xtl = xtail_pool.tile([I_TAIL, F], f32)
                    nc.sync.dma_start(out=xtl, in_=xt[I_MAIN:, t * BC:(t + 2) * BC])
                if t % KT == 0:
                    spk_stage = stage_pool.tile([128, KT * BQ], f32, tag="spk")
                    mem_stage = stage_pool.tile([128, KT * BQ], f32, tag="mem")

                half = (t % 2) * BC
                ps = psum_pool.tile([128, BQ], f32)
                for c in range(7):
                    if c < 6:
                        lhsT = wt_sb[:, c, :]
                        src, base = xm, half
                    else:
                        lhsT = wt_sb[:I_TAIL, 6, :]
                        src, base = xtl, (t % 2) * BC
                    for j in range(4):
                        if c < 6:
                            rhs = src[:, c, base + j * BQ: base + (j + 1) * BQ]
                        else:
                            rhs = src[:, base + j * BQ: base + (j + 1) * BQ]
                        nc.tensor.matmul(
                            ps[32 * j:32 * j + 32, :],
                            lhsT=lhsT,
                            rhs=rhs,
                            start=(c == 0),
                            stop=(c == 6),
                            tile_position=(0, 32 * j),
                        )

                # mem = (mem * BETA) + cur_t   (same op order as reference)
                nc.vector.scalar_tensor_tensor(
                    out=mem_state, in0=mem_state, scalar=BETA, in1=ps,
                    op0=AL.mult, op1=AL.add,
                )
                # mem -= spk_{t-1} * THRESHOLD (THRESHOLD == 1)
                nc.vector.tensor_sub(mem_state, mem_state, spk_prev)
                off = (t % KT) * BQ
                spk_sl = spk_stage[:, off:off + BQ]
                nc.vector.tensor_scalar(
                    out=spk_sl, in0=mem_state, scalar1=THRESHOLD,
                    scalar2=None, op0=AL.is_gt,
                )
                mem_sl = mem_stage[:, off:off + BQ]
                nc.scalar.copy(mem_sl, mem_state)
                spk_prev = spk_sl

                if t % KT == KT - 1:
                    t0 = t - (KT - 1)
                    for j in range(4):
                        nc.scalar.dma_start(
                            out=spk[j, :, t0:t0 + KT, :],
                            in_=spk_stage[32 * j:32 * j + O, :].rearrange(
                                "p (k q) -> p k q", k=KT),
                        )
                        nc.scalar.dma_start(
                            out=mem[j, :, t0:t0 + KT, :],
                            in_=mem_stage[32 * j:32 * j + O, :].rearrange(
                                "p (k q) -> p k q", k=KT),
                        )

    nc.finalize()
    return nc


_NC = None


def _get_nc():
    global _NC
    if _NC is None:
        _NC = _build_nc()
    return _NC


def _quantize_w(W):
    W32 = np.asarray(W, dtype=np.float32)
    q = np.round(W32 * np.float32(4096.0))
    q = np.clip(q, np.float32(-32768.0), np.float32(32767.0)) / np.float32(4096.0)
    # straight-through forward value, replicated bit-exactly: w + (q - w)
    return (W32 + (q - W32)).astype(np.float32)


def kernel(x, W, _run_opts=None):
    x = np.asarray(x, dtype=np.float32)
    W = np.asarray(W, dtype=np.float32)
    assert x.shape == (T, B, I) and W.shape == (O, I)

    wt = np.ascontiguousarray(_quantize_w(W).T)  # [784, 10]

    in_maps = []
    for c in range(N_CORES):
        xc = x[:, c * BC:(c + 1) * BC, :]                      # [30, 1024, 784]
        xt_c = np.ascontiguousarray(xc.transpose(2, 0, 1))     # [784, 30, 1024]
        in_maps.append({"xt": xt_c.reshape(I, N), "wt": wt})

    nc = _get_nc()
    run_opts = dict(_run_opts or {})
    res = run_bass_kernel_spmd(nc, in_maps, core_ids=list(range(N_CORES)), **run_opts)

    spk_full = np.empty((T, B, O), dtype=np.float32)
    mem_full = np.empty((T, B, O), dtype=np.float32)
    for c in range(N_CORES):
        # device layout [4, 10, T, 256] -> [T, 1024, 10]
        s = res.results[c]["spk"].transpose(2, 0, 3, 1).reshape(T, BC, O)
        m = res.results[c]["mem"].transpose(2, 0, 3, 1).reshape(T, BC, O)
        spk_full[:, c * BC:(c + 1) * BC, :] = s
        mem_full[:, c * BC:(c + 1) * BC, :] = m

    if _run_opts is not None:
        kernel.last_result = res
    return spk_full, mem_full
